# revision 30
# baseline (speedup 1.0000x reference)
"""Trainium2 Bass kernel for nn_EnhancedCardAwarePolicy.

Strategy: pure data-parallel across 8 NeuronCores (batch 16384 -> 2048/core).

Key algebraic simplifications (value-preserving vs the reference):
  * The hand self-attention MHA is dead code: the cross-attention that
    consumes it has sequence length 1, so its softmax is identically 1 and
    its output is independent of the query.  hand_ctx reduces to
        (8 / max(hand_size,1)) * (enemy_emb @ he_wv @ he_wo + he_bv @ he_wo + he_bo)
  * Card encodings are pure functions of the card index -> fold the
    embedding tables into one [54, 32] table, folded through downstream
    linears; the enemy-card path becomes matmuls against one-hot columns
    built on-device.  The 1/hand_size scale is folded into a second,
    scaled one-hot so the whole hand path is a single fused matmul.
  * cx_w3 has no nonlinearity after it, so it is folded into all three of
    its consumers (action-MLP layer 1, action-type head layer 1); the ctx
    activation is never materialized.
  * strat_ctx's second linear layer is folded into cx_w1.
  * The per-action tables are folded into per-action bias vectors on the
    host.  softmax+bonus is computed unnormalized via a tiny matmul.

All matmuls run in fp16 (fp32 matmuls cost 4 cycles/row on TRN2's PE,
fp16 cost 1); accumulation stays fp32 in PSUM.  Per-batch scalar stats
travel batch-major through one small DMA and are rotated into rows with
PE transposes.  PSUM->SBUF evictions are split across the Scalar, Vector
and GpSimd(Pool) engines.
"""

import numpy as np
from contextlib import ExitStack

B = 16384
NCORES = 8
BC = B // NCORES          # 2048 batch rows per core
NCH = 4                   # chunks per core
N = BC // NCH             # 512 batch columns per chunk
A = 30                    # real actions
NP = 15                   # action pairs
E = 32
HID = 128
NS = 22                   # stats per batch element (see layout below)

_cache = {}


# ---------------------------------------------------------------------------
# host-side folding
# ---------------------------------------------------------------------------

def _card_table(val_emb, suit_emb, type_emb):
    """[54, 32] full card encoding table, matching _encode_cards."""
    c = np.arange(54)
    invalid = (c == 0) | (c == 53)
    v = np.where(invalid, 0, (c - 1) % 13 + 1)
    s = np.where(invalid, 0, (c - 1) // 13 + 1)
    ce = np.concatenate([val_emb[v], suit_emb[s]], axis=-1)          # [54, 32]
    ct = np.where(v == 11, 1, np.where(v == 12, 2, np.where(v == 13, 3, 0)))
    te = type_emb[ct]                                                # [54, 8]
    pad = np.zeros((54, E - te.shape[-1]), np.float32)
    return (ce + np.concatenate([te, pad], axis=-1)).astype(np.float32)


def _action_fold(ac, card_emb, ce_w1, ce_b1, ce_w2, ce_b2,
                 as_w1, as_b1, as_b3):
    """Per-action biases + bonus matrix from action_card_indices [30, 4]."""
    ac = np.asarray(ac, np.int64)
    mask = ac != 0
    combo_size = mask.sum(1).astype(np.float32)
    values = np.where(mask, (ac - 1) % 13 + 1, 0)
    has_valid = mask.any(1)
    fidx = np.argmax(mask, axis=1)
    fv = values[np.arange(ac.shape[0]), fidx]
    same = np.where(mask, values == fv[:, None], True).all(1).astype(np.float32)
    vf = values.astype(np.float32)
    attack = np.where(values == 1, 1.0,
             np.where(values == 11, 10.0,
             np.where(values == 12, 15.0,
             np.where(values == 13, 20.0, vf))))
    total = (attack * mask).sum(1).astype(np.float32)
    suits = np.where(mask, (ac - 1) // 13 + 1, 0)
    uniq = sum((suits == s).any(1) for s in (1, 2, 3, 4)).astype(np.float32)
    ace = ((values == 1) & mask).any(1).astype(np.float32)
    valid = ((combo_size <= 4.0) & ((same > 0) | (ace > 0))).astype(np.float32)
    feats = np.stack([combo_size, same, total, uniq, ace, valid], 1)
    feats = np.where(has_valid[:, None], feats, 0.0).astype(np.float32)

    emb = card_emb[ac]                                   # [30, 4, 32]
    m = mask.astype(np.float32)[..., None]
    cnt = np.maximum(m.sum(1), 1.0)
    act_emb = (emb * m).sum(1) / cnt
    act_emb = np.where(has_valid[:, None], act_emb, 0.0).astype(np.float32)
    combo_enc = np.maximum(feats @ ce_w1 + ce_b1, 0.0) @ ce_w2 + ce_b2

    action_bias = act_emb @ as_w1[HID:HID + E] + combo_enc @ as_w1[HID + E:] + as_b1

    strength = feats[:, 2] / 20.0
    b3 = float(as_b3[0])
    Bm1 = np.zeros((4, 34), np.float32)
    for a in range(A):
        if has_valid[a]:
            col = np.array([strength[a], 1.0 - strength[a], 0.0, 0.0])
        else:
            col = np.array([0.0, 0.0, 0.0, 2.0])
        Bm1[:, a] = col + b3
    Bm1[:, 32] = 1.0                                     # denominator column
    return action_bias, Bm1                              # [30, 64], [4, 34]


def _prep(inputs):
    """Fold weights, build per-core input maps. Returns (in_maps, consts)."""
    f32 = lambda x: np.ascontiguousarray(np.asarray(x), dtype=np.float32)
    hc = np.asarray(inputs["hand_cards"])        # [B, 8] int
    ec = np.asarray(inputs["enemy_card"])        # [B]
    hs = np.asarray(inputs["hand_size"])         # [B]
    gs = f32(inputs["game_state"])               # [B, 10]
    dc = f32(inputs["discard_pile_cards"])       # [B, 54]

    card_emb = _card_table(f32(inputs["val_emb"]), f32(inputs["suit_emb"]),
                           f32(inputs["type_emb"]))
    card_emb1 = np.concatenate([card_emb, np.ones((54, 1), np.float32)], 1)

    he_wv, he_bv = f32(inputs["he_wv"]), f32(inputs["he_bv"])
    he_wo, he_bo = f32(inputs["he_wo"]), f32(inputs["he_bo"])
    Mc = np.concatenate([he_wv @ he_wo, (he_bv @ he_wo + he_bo)[None]], 0)  # [33,32]
    A0s = 8.0 * (card_emb1 @ Mc)                                   # [54, 32]

    cx_w1, cx_b1 = f32(inputs["cx_w1"]), f32(inputs["cx_b1"])
    W1h = np.ascontiguousarray(cx_w1[0:E])                         # [32, 128]
    A2 = card_emb @ cx_w1[E:2 * E]                                 # [54, 128]
    W1s = cx_w1[2 * E:2 * E + 32]                                  # [32, 128]
    W1d = np.ascontiguousarray(cx_w1[2 * E + 32:])                 # [54, 128]
    se_w1, se_b1 = f32(inputs["se_w1"]).copy(), f32(inputs["se_b1"])
    se_w2, se_b2 = f32(inputs["se_w2"]), f32(inputs["se_b2"])
    U = se_w2 @ W1s                                                # [64, 128]
    b1f = cx_b1 + se_b2 @ W1s                                      # [128]

    # stats layout (NS=22 per batch element):
    #  0 ec | 1 r=1/hs (device) | 2 hs | 3..12 gs | 13 aces | 14 faces
    #  | 15 low | 16..19 su1..4 | 20 hvr | 21 sdiv_cnt
    sew1 = np.zeros((NS, 64), np.float32)
    sew1[2] = se_w1[10]
    sew1[3:13] = se_w1[0:10]
    sew1[13:20] = se_w1[11:18]
    sew1[20] = se_w1[18]
    sew1[21] = se_w1[19] / 4.0    # device computes suit-diversity count 0..4

    cx_w2, cx_b2 = f32(inputs["cx_w2"]), f32(inputs["cx_b2"])
    cx_w3, cx_b3 = f32(inputs["cx_w3"]), f32(inputs["cx_b3"])
    atc_w1, atc_b1 = f32(inputs["atc_w1"]), f32(inputs["atc_b1"])
    atc_w2, atc_b2 = f32(inputs["atc_w2"]), f32(inputs["atc_b2"])
    as_w1, as_b1 = f32(inputs["as_w1"]), f32(inputs["as_b1"])
    as_w2, as_b2 = f32(inputs["as_w2"]), f32(inputs["as_b2"])
    as_w3, as_b3 = f32(inputs["as_w3"]), f32(inputs["as_b3"])
    ab, Bm1 = _action_fold(inputs["action_card_indices"], card_emb,
                           f32(inputs["ce_w1"]), f32(inputs["ce_b1"]),
                           f32(inputs["ce_w2"]), f32(inputs["ce_b2"]),
                           as_w1, as_b1, as_b3)

    # fold cx_w3 (no nonlinearity after it) into its consumers
    W1c = as_w1[:HID]                                              # [128, 64]
    W1cd = np.concatenate([W1c, W1c], 1)                           # [128, 128]
    W1cdf = cx_w3 @ W1cd                                           # [128, 128]
    ab_f = ab + (cx_b3 @ W1c)[None]                                # [30, 64]
    atw1f = cx_w3 @ atc_w1                                         # [128, 64]
    atb1f = atc_b1 + cx_b3 @ atc_w1                                # [64]

    A0sW1h = A0s @ W1h                                             # [54, 128]
    # one-hot stack: rows 0:54 scaled one-hot (r * onehot), rows 54:64 zero,
    # rows 64:118 plain one-hot
    K108 = np.concatenate([A0sW1h, np.zeros((10, HID), np.float32), A2], 0)
    K118 = np.concatenate([U, W1d], 0)                             # [118, 128]
    # broadcast selectors: rhs is sh_in[0:2] = [ec; r] rows
    ones_ec = np.zeros((2, 118), np.float32)
    ones_ec[0, :] = 1.0            # ec broadcast to all 118 rows
    ones_r = np.zeros((2, 64), np.float32)
    ones_r[1, :] = 1.0             # r broadcast to rows 0:64

    abp = np.zeros((128, NP), np.float32)
    for p in range(NP):
        abp[0:64, p] = ab_f[2 * p]
        abp[64:128, p] = ab_f[2 * p + 1]
    W2blk = np.zeros((128, 64), np.float32)
    W2blk[0:64, 0:32] = as_w2
    W2blk[64:128, 32:64] = as_w2
    b2q = np.tile(as_b2, 4)[:, None].astype(np.float32)            # [128, 1]
    w3blk = np.zeros((128, 4), np.float32)
    for i in range(4):
        w3blk[32 * i:32 * i + 32, i] = as_w3[:, 0]

    # ---- fp16 const blob: [128, C16] with named column spans
    f16_parts = [
        ("ident", np.eye(128, dtype=np.float32)),
        ("K108", K108), ("K118", K118), ("sew1", sew1),
        ("cxw2", cx_w2), ("W1cdf", W1cdf), ("atw1f", atw1f),
        ("atw2", atc_w2), ("W2blk", W2blk), ("w3blk", w3blk),
        ("Bm1", Bm1),
        ("ones_ec", ones_ec), ("ones_r", ones_r),
    ]
    off16 = {}
    cols = 0
    for nm, arr in f16_parts:
        off16[nm] = (cols, arr.shape[0], arr.shape[1])
        cols += arr.shape[1]
    blob16 = np.zeros((128, cols), np.float16)
    for nm, arr in f16_parts:
        o, p, f = off16[nm]
        blob16[0:p, o:o + f] = arr.astype(np.float16)

    # ---- fp32 const blob (biases etc.)
    # iota column: rows 0:64 hold 0..63 (values 54..63 never match a card so
    # the scaled one-hot rows 54:64 are naturally zero); rows 64:118 hold
    # 0..53 again for the base-64-aligned plain one-hot compare.
    iota2 = np.zeros((118, 1), np.float32)
    iota2[0:64, 0] = np.arange(64)
    iota2[64:118, 0] = np.arange(54)
    f32_parts = [
        ("iota2", iota2),
        ("seb1", se_b1[:, None]), ("b1f", b1f[:, None]),
        ("cxb2", cx_b2[:, None]),
        ("atb1f", atb1f[:, None]), ("atb2", atc_b2[:, None]),
        ("b2q", b2q), ("abp", abp),
        ("sgb", np.tile(np.array([[-13.5, -26.5, -39.5]], np.float32),
                        (128, 1))),
    ]
    off32 = {}
    cols = 0
    for nm, arr in f32_parts:
        off32[nm] = (cols, arr.shape[0], arr.shape[1])
        cols += arr.shape[1]
    blob32 = np.zeros((128, cols), np.float32)
    for nm, arr in f32_parts:
        o, p, f = off32[nm]
        blob32[0:p, o:o + f] = arr

    consts = {"off16": off16, "off32": off32,
              "c16": blob16.shape[1], "c32": blob32.shape[1]}

    in_maps = []
    for i in range(NCORES):
        sl = slice(i * BC, (i + 1) * BC)
        m = {"blob16": blob16, "blob32": blob32}
        m["hcS"] = np.ascontiguousarray(
            hc[sl].astype(np.float32).reshape(16, 128, 8)
            .transpose(1, 0, 2).reshape(128, 128))
        stats = np.zeros((128, 16, 13), np.float32)
        stats[:, :, 0] = ec[sl].astype(np.float32).reshape(16, 128).T
        stats[:, :, 2] = hs[sl].astype(np.float32).reshape(16, 128).T
        stats[:, :, 3:13] = gs[sl].reshape(16, 128, 10).transpose(1, 0, 2)
        m["statsBM"] = np.ascontiguousarray(
            stats.reshape(128, 208).astype(np.float16))
        m["dcT"] = np.ascontiguousarray(dc[sl].T.astype(np.float16))
        m["rT"] = np.ascontiguousarray(
            (1.0 / np.maximum(hs[sl].astype(np.float32), 1.0))
            .astype(np.float16)[None, :])
        in_maps.append(m)
    return in_maps, consts


# ---------------------------------------------------------------------------
# device program
# ---------------------------------------------------------------------------

def _build(consts):
    import concourse.bass as bass
    import concourse.tile as tile
    import concourse.mybir as mybir
    from concourse import bacc

    dt = mybir.dt.float32
    dth = mybir.dt.float16
    AF = mybir.ActivationFunctionType
    OP = mybir.AluOpType
    AX = mybir.AxisListType

    nc = bacc.Bacc("TRN2", target_bir_lowering=False, debug=False,
                   enable_asserts=False, num_devices=NCORES)

    din = {}
    din["blob16"] = nc.dram_tensor("blob16", [128, consts["c16"]], dth,
                                   kind="ExternalInput").ap()
    din["blob32"] = nc.dram_tensor("blob32", [128, consts["c32"]], dt,
                                   kind="ExternalInput").ap()
    din["hcS"] = nc.dram_tensor("hcS", [128, 128], dt,
                                kind="ExternalInput").ap()
    din["statsBM"] = nc.dram_tensor("statsBM", [128, 208], dth,
                                    kind="ExternalInput").ap()
    din["dcT"] = nc.dram_tensor("dcT", [54, BC], dth,
                                kind="ExternalInput").ap()
    din["rT"] = nc.dram_tensor("rT", [1, BC], dth,
                               kind="ExternalInput").ap()
    out_d = nc.dram_tensor("out", [BC, A], dt, kind="ExternalOutput").ap()
    # out rows b = 512*n + 128*s + p  ->  [n][p, s, a]
    out_r = out_d.rearrange("(n s p) a -> n p s a", n=NCH, s=4, p=128)

    with tile.TileContext(nc) as tc, ExitStack() as ctx:
        cpool = ctx.enter_context(tc.tile_pool(name="consts", bufs=1))
        core = ctx.enter_context(tc.tile_pool(name="core", bufs=1))
        work = ctx.enter_context(tc.tile_pool(name="work", bufs=3))
        s1p = ctx.enter_context(tc.tile_pool(name="s1p", bufs=2))
        s2p = ctx.enter_context(tc.tile_pool(name="s2p", bufs=2))
        fout = ctx.enter_context(tc.tile_pool(name="fout", bufs=2))
        ps_f = ctx.enter_context(tc.tile_pool(name="ps_f", bufs=2, space="PSUM"))
        ps_c = ctx.enter_context(tc.tile_pool(name="ps_c", bufs=1, space="PSUM"))
        ps_z = ctx.enter_context(tc.tile_pool(name="ps_z", bufs=2, space="PSUM"))
        ps_o = ctx.enter_context(tc.tile_pool(name="ps_o", bufs=1, space="PSUM"))

        # ---- constants into SBUF (2 blob DMAs) + core inputs
        # order matters: hcS + statsBM first so preamble compute starts early
        hcS = core.tile([128, 128], dt, tag="hcS")
        nc.gpsimd.dma_start(hcS[:], din["hcS"])
        S = core.tile([128, 16 * NS], dth, tag="S")
        S22 = S.rearrange("p (g k) -> p g k", k=NS)
        nc.sync.dma_start(S22[:, :, 0:13],
                          din["statsBM"].rearrange("p (g k) -> p g k", k=13))
        c16 = cpool.tile([128, consts["c16"]], dth, tag="c16", name="c16")
        nc.sync.dma_start(c16[:], din["blob16"])
        c32 = cpool.tile([128, consts["c32"]], dt, tag="c32", name="c32")
        nc.gpsimd.dma_start(c32[:], din["blob32"])

        def c16ap(nm):
            o, p, f = consts["off16"][nm]
            return c16[0:p, o:o + f]

        def c32ap(nm):
            o, p, f = consts["off32"][nm]
            return c32[0:p, o:o + f]

        # ---- per-card features via the sign trick:
        #   SS = sign(hc-13.5)+sign(hc-26.5)+sign(hc-39.5)  in {-3,-1,1,3}
        #   raw = hc - 6.5*SS = (value 1..13 or 0 for pad) + 19.5
        # threshold tests against raw/SS replace the explicit value/suit
        # decomposition; the 3 sign ops run on the otherwise idle Scalar.
        v = nc.vector
        sc = nc.scalar
        ft = {k: core.tile([128, 128], dt, tag=f"ft_{k}", name=f"ft_{k}")
              for k in ("sa", "sb", "sc", "ss", "raw", "mask",
                        "ace", "face", "lowd", "low", "su1", "su2", "su3",
                        "su4")}
        sgb = c32ap("sgb")
        sc.activation(ft["sa"][:], hcS[:], AF.Sign, bias=sgb[:, 0:1])
        sc.activation(ft["sb"][:], hcS[:], AF.Sign, bias=sgb[:, 1:2])
        sc.activation(ft["sc"][:], hcS[:], AF.Sign, bias=sgb[:, 2:3])
        v.tensor_tensor(ft["ss"][:], ft["sa"][:], ft["sb"][:], OP.add)
        v.tensor_tensor(ft["ss"][:], ft["ss"][:], ft["sc"][:], OP.add)
        v.scalar_tensor_tensor(ft["raw"][:], ft["ss"][:], -6.5, hcS[:],
                               OP.mult, OP.add)
        v.tensor_scalar(ft["mask"][:], hcS[:], 0.5, None, OP.is_ge)
        v.tensor_scalar(ft["ace"][:], ft["raw"][:], 20.5, None, OP.is_equal)
        v.tensor_scalar(ft["face"][:], ft["raw"][:], 30.0, None, OP.is_ge)
        v.tensor_scalar(ft["lowd"][:], ft["raw"][:], 21.0, None, OP.is_ge)
        v.scalar_tensor_tensor(ft["low"][:], ft["raw"][:], 26.0, ft["lowd"][:],
                               OP.is_le, OP.mult)
        for k, s in (("su1", -3.0), ("su2", -1.0), ("su3", 1.0), ("su4", 3.0)):
            v.scalar_tensor_tensor(ft[k][:], ft["ss"][:], s, ft["mask"][:],
                                   OP.is_equal, OP.mult)

        # ---- reduce 8 cards -> per-batch stats into S columns
        # (sums of <=8 small integers: exact in fp16)
        with nc.allow_low_precision(reason="stat sums are small exact ints"):
            for k, ki in (("ace", 13), ("face", 14), ("low", 15),
                          ("su1", 16), ("su2", 17), ("su3", 18), ("su4", 19)):
                src = ft[k].rearrange("p (j c) -> p j c", c=8)
                v.tensor_reduce(S22[:, :, ki], src, AX.X, OP.add)

        # ---- r = 1/hs (hs >= 1 always), hvr = faces*r, sdiv count
        rmax = core.tile([128, 16], dt, tag="rmax")
        v.tensor_scalar(rmax[:], S22[:, :, 2], 1.0, None, OP.max)
        rinv = core.tile([128, 16], dt, tag="rinv")
        v.reciprocal(rinv[:], rmax[:])
        v.tensor_tensor(S22[:, :, 20], S22[:, :, 14], rinv[:], OP.mult)
        ge = [core.tile([128, 16], dt, tag=f"ge{k}", name=f"ge{k}")
              for k in range(4)]
        for k in range(4):
            v.tensor_scalar(ge[k][:], S22[:, :, 16 + k], 1.0, None, OP.min)
        v.tensor_tensor(ge[0][:], ge[0][:], ge[1][:], OP.add)
        v.tensor_tensor(ge[2][:], ge[2][:], ge[3][:], OP.add)
        v.tensor_tensor(S22[:, :, 21], ge[0][:], ge[2][:], OP.add)

        ident = c16ap("ident")
        st = {}     # per-chunk tiles shared between front and action stages

        def emit_front(n):
            cols = slice(N * n, N * (n + 1))

            # dcT chunk DMA lands directly in the z1 moving tile
            z1db = work.tile([118, N], dth, tag="z1db", name=f"z1db_{n}")
            (nc.sync if n % 2 == 0 else nc.gpsimd).dma_start(
                z1db[64:118, :], din["dcT"][:, cols])
            # r broadcast to 64 partitions via stride-0 DRAM read
            rbs = work.tile([64, N], dth, tag="rbs", name=f"rbs_{n}")
            nc.gpsimd.dma_start(rbs[:], bass.AP(din["rT"].tensor,
                                                din["rT"].offset + N * n,
                                                [[0, 64], [1, N]]))

            # rotate per-batch stats into rows: 4 transposes of [128, 22]
            scalT = ps_f.tile([128, N], dth, tag="fe", name=f"scalT_{n}")
            for s in range(4):
                nc.tensor.transpose(scalT[0:NS, 128 * s:128 * (s + 1)],
                                    S22[:, 4 * n + s, :], ident)
            sh_in = work.tile([NS, N], dth, tag="sh_in", name=f"sh_in_{n}")
            v.tensor_scalar(sh_in[:], scalT[0:NS, :], 0.0, None, OP.add)

            # strat hidden layer -> z1db rows 0:64
            shp = ps_f.tile([128, N], dt, tag="fe", name=f"shp_{n}")
            nc.tensor.matmul(shp[0:64, :], c16ap("sew1"), sh_in[:],
                             start=True, stop=True)
            sc.activation(z1db[0:64, :], shp[0:64, :], AF.Relu,
                          bias=c32ap("seb1")[:, 0:1])

            # enemy one-hot (r-scaled rows 0:64, plain rows 64:118)
            ecb = ps_f.tile([128, N], dt, tag="fe", name=f"ecb_{n}")
            nc.tensor.matmul(ecb[0:118, :], c16ap("ones_ec"), sh_in[0:2, :],
                             start=True, stop=True)
            ohh = work.tile([118, N], dth, tag="ohh", name=f"ohh_{n}")
            v.scalar_tensor_tensor(ohh[0:64, :], ecb[0:64, :],
                                   c32ap("iota2")[0:64, 0:1], rbs[:],
                                   OP.is_equal, OP.mult)
            v.tensor_scalar(ohh[64:118, :], ecb[64:118, :],
                            c32ap("iota2")[64:118, 0:1], None, OP.is_equal)

            # z1 = K108^T [ohr; oh] + K118^T [sh; dc] + b1f
            z1 = ps_f.tile([128, N], dt, tag="fe", name=f"z1_{n}")
            nc.tensor.matmul(z1[:], c16ap("K108"), ohh[:], start=True,
                             stop=False)
            nc.tensor.matmul(z1[:], c16ap("K118"), z1db[:], start=False,
                             stop=True)
            h1 = work.tile([128, N], dth, tag="h1", name=f"h1_{n}")
            sc.activation(h1[:], z1[:], AF.Relu, bias=c32ap("b1f")[:, 0:1])

            h2p = ps_f.tile([128, N], dt, tag="fe", name=f"h2p_{n}")
            nc.tensor.matmul(h2p[:], c16ap("cxw2"), h1[:], start=True,
                             stop=True)
            h2 = work.tile([128, N], dth, tag="h2", name=f"h2_{n}")
            v.tensor_scalar(h2[:], h2p[:], c32ap("cxb2")[:, 0:1], 0.0,
                            OP.add, OP.max)

            # action-type probs (unnormalized exp), cxw3 folded in
            tphp = ps_f.tile([128, N], dt, tag="fe", name=f"tphp_{n}")
            nc.tensor.matmul(tphp[0:64, :], c16ap("atw1f"), h2[:],
                             start=True, stop=True)
            tph = work.tile([64, N], dth, tag="tph", name=f"tph_{n}")
            sc.activation(tph[:], tphp[0:64, :], AF.Relu,
                          bias=c32ap("atb1f")[:, 0:1])
            tlp = ps_f.tile([128, N], dt, tag="fe", name=f"tlp_{n}")
            nc.tensor.matmul(tlp[0:4, :], c16ap("atw2"), tph[:],
                             start=True, stop=True)
            expl = work.tile([4, N], dth, tag="expl", name=f"expl_{n}")
            sc.activation(expl[:], tlp[0:4, :], AF.Exp,
                          bias=c32ap("atb2")[:, 0:1])
            st[n] = {"h2": h2, "expl": expl}

        def emit_ctx1d(n):
            # action-MLP layer-1 preactivation (shared across actions);
            # emitted at the tail of the previous action stage so the
            # single-buffered PSUM bank is free by the time it runs
            ctx1d = ps_c.tile([128, N], dt, tag="ctx1d", name=f"ctx1d_{n}")
            nc.tensor.matmul(ctx1d[:], c16ap("W1cdf"), st[n]["h2"][:],
                             start=True, stop=True)
            st[n]["ctx1d"] = ctx1d

        def emit_action(n):
            ctx1d = st[n]["ctx1d"]
            expl = st[n]["expl"]

            # per-pair relu evictions, split Scalar/Vector
            abp = c32ap("abp")
            s1t = []
            for p in range(NP):
                t1 = s1p.tile([128, N], dth, tag=f"s1_{p}", name=f"s1_{n}_{p}")
                if p % 2 == 0:
                    sc.activation(t1[:], ctx1d[:], AF.Relu,
                                  bias=abp[:, p:p + 1])
                else:
                    v.tensor_scalar(t1[:], ctx1d[:], abp[:, p:p + 1], 0.0,
                                    OP.add, OP.max)
                s1t.append(t1)

            # score cols 0:128, numer cols 128:264 -- one PSUM bank
            sn = ps_o.tile([128, 264], dt, tag="fin", name=f"sn_{n}")
            score = sn[:, 0:128]
            W2 = c16ap("W2blk")
            w3 = c16ap("w3blk")
            b2q1 = c32ap("b2q")[:, 0:1]
            # z2 double-width tiles: two q-groups (8 actions) per PSUM pair,
            # evicted in one 1024-column instruction (same bias everywhere)
            for qq in range(4):
                z2w = ps_z.tile([128, 2 * N], dt, tag="z2",
                                name=f"z2_{n}_{qq}")
                for half in range(2):
                    q = 2 * qq + half
                    npair = 2 if q < 7 else 1
                    for j in range(npair):
                        nc.tensor.matmul(
                            z2w[64 * j:64 * j + 64, N * half:N * (half + 1)],
                            W2, s1t[2 * q + j][:], start=True, stop=True)
                t = s2p.tile([128, 2 * N], dth, tag=f"s2_{qq % 2}",
                             name=f"s2_{n}_{qq}")
                if qq < 3:
                    if qq % 2 == 0:
                        v.tensor_scalar(t[:], z2w[:], b2q1, 0.0,
                                        OP.add, OP.max)
                    else:
                        sc.activation(t[:], z2w[:], AF.Relu, bias=b2q1)
                else:
                    # last group: q=7 wrote only rows 0:64 of the upper half
                    sc.activation(t[:, 0:N], z2w[:, 0:N], AF.Relu, bias=b2q1)
                    v.tensor_scalar(t[0:64, N:2 * N], z2w[0:64, N:2 * N],
                                    b2q1[0:64, :], 0.0, OP.add, OP.max)
                for half in range(2):
                    q = 2 * qq + half
                    npair = 2 if q < 7 else 1
                    rows = 64 * npair
                    na = 2 * npair
                    for s in range(4):
                        nc.tensor.matmul(
                            score[:, 32 * s + 4 * q:32 * s + 4 * q + na],
                            t[0:rows,
                              N * half + 128 * s:N * half + 128 * (s + 1)],
                            w3[0:rows, 0:na], start=True, stop=True)

            numer = sn[:, 128:264]
            for s in range(4):
                nc.tensor.matmul(numer[:, 34 * s:34 * (s + 1)],
                                 expl[:, 128 * s:128 * (s + 1)],
                                 c16ap("Bm1"), start=True, stop=True)
            if n + 1 < NCH:
                emit_ctx1d(n + 1)

            recipT = fout.tile([128, 4], dt, tag="recip", name=f"recip_{n}")
            den = numer.rearrange("p (s c) -> p s c", c=34)[:, :, 32]
            v.reciprocal(recipT[:], den)
            tmp = fout.tile([128, 120], dt, tag="tmp", name=f"tmp_{n}")
            for s in range(4):
                v.tensor_scalar(tmp[:, 30 * s:30 * (s + 1)],
                                numer[:, 34 * s:34 * s + 30],
                                recipT[:, s:s + 1], None, OP.mult)
            outT = fout.tile([128, 120], dt, tag="outT", name=f"outT_{n}")
            sc_ap = score.rearrange("p (s c) -> p s c", c=32)[:, :, 0:30]
            v.tensor_tensor(outT.rearrange("p (s c) -> p s c", c=30),
                            tmp.rearrange("p (s c) -> p s c", c=30),
                            sc_ap, OP.add)
            (nc.sync if n % 2 == 0 else nc.gpsimd).dma_start(
                out_r[n], outT.rearrange("p (s c) -> p s c", c=30))
            del st[n]

        # ---- 2-stage software pipeline: chunk n+1's front is emitted
        # before chunk n's action phase so its latency chain overlaps
        for n in range(NCH):
            emit_front(n)
            if n == 0:
                emit_ctx1d(0)
            if n >= 1:
                emit_action(n - 1)
        emit_action(NCH - 1)

    nc.compile()
    return nc


def _get_program(consts):
    key = "prog"
    if key not in _cache:
        _cache[key] = _build(consts)
    return _cache[key]


def kernel(**inputs):
    in_maps, consts = _prep(inputs)
    nc = _get_program(consts)
    from concourse.bass_utils import run_bass_kernel_spmd
    res = run_bass_kernel_spmd(nc, in_maps, core_ids=list(range(NCORES)))
    out = np.concatenate([res.results[i]["out"] for i in range(NCORES)], 0)
    return out.astype(np.float32)
